# revision 29
# baseline (speedup 1.0000x reference)
"""Trainium2 kernel for nn_ConservationOfFeatureSimilarity.

Math (see reference): with xn = row-normalized feature embeddings (M, 256) and
zn = row-normalized frozen embeddings (M, 768), M = B*N = 3136:

  feat_sim  = xn @ xn.T        (M, M)
  frozen_sim= zn @ zn.T        (M, M)
  ranking   = triu+ * (feat-frozen) * [cls_i != cls_j] * [pidx_i == pidx_j] * mps_i*mps_j
  top5      = top_k(ranking.flat, 5);  sel rows/cols
  out       = mean |feat_sim[sel] - frozen_sim[sel]|  over (5, 2, M)
            = (sum over the 10 selected row indices of S[r]) / (10*M)
  where S_i = sum_j |feat_sim[i,j] - frozen_sim[i,j]|.

Device (8 NeuronCores): the dense O(M^2 * D) part — S row sums. |diff| is
symmetric, so only upper-triangular blocks of the (32 x 8) tile grid are
computed: each computed block contributes row sums and, for strictly-upper
blocks, column sums for the mirrored block (ones-masked matmul on |d|).
Per-core work is SPMD-uniform: core c owns row-tiles {8t+c : t=0..3} (98 rows
each) and slot t computes col-blocks J >= 2t (392 cols each); per-core 0/1
mask vectors (data, not code) select which blocks feed the column-sum
accumulator, and the host drops the few below-diagonal rowsum partials.

The tile difference feat-frozen is accumulated directly in PSUM via 4 chained
fp8(e4m3) DoubleRow matmuls (each contracting 2 128-chunks: 1 pair for
+xn.xn, 3 pairs for (-zn).zn using host-negated row slices; inputs pre-scaled
by 16 so quantization stays in fp8 normal range, descaled by 256 on host).
A single fused DVE tensor_scalar(abs_max, accum_out) produces |d| in SBUF
(for the masked column-sum matmul on TensorE) and the row sums in one pass.

Host: normalization/transposes, fp8 quantization, prototype argmax, the
top-5 search (ranking is nonzero only for same-argmax-prototype pairs:
~25K of the 9.8M pairs, evaluated sparsely in numpy in exact arithmetic,
so selection is unaffected by fp8), and the final scalar combine.
"""

import sys

if "/opt/trn_rl_repo" not in sys.path:
    sys.path.insert(0, "/opt/trn_rl_repo")

import numpy as np
import ml_dtypes

BF16 = ml_dtypes.bfloat16
FP8 = ml_dtypes.float8_e4m3

B, N, D, NF, P = 16, 196, 768, 256, 200
M = B * N                      # 3136
NCORES = 8
RT = 98                        # row tile height
RTP = 112                      # row tile slot pitch in SBUF (16B aligned)
NSLOT = 4                      # row tiles per core (slot t -> global tile 8t+c)
CB = 392                       # col block width
CBP = 400                      # col block pitch in SBUF (16B aligned)
NJ = 8                         # col blocks
NK = 8                         # 128-chunks: 2 feat + 6 frozen
NPAIR = 4                      # DoubleRow chunk pairs
K_ = 5
GAMMA = 1.0
EPS = 1e-8
FP8_SCALE = 16.0               # inputs pre-scaled; |d| scaled by 256

# program-order block list: (t, J) with J >= 2t. Descending J so the first
# bands DMA'd in feed the largest block groups (PE consumption stays behind
# the DMA arrival curve). The four diagonal blocks (J == 2t) never feed the
# column-sum accumulator (their cmask is all-zero for every core), so they
# are deferred to the end of the stream where they use the short fused
# Abs+accum path with no colsum dependency chain behind them.
JORDER = list(range(NJ - 1, -1, -1))
BLOCKS = [(t, J) for J in JORDER for t in range(NSLOT) if J > 2 * t]
DIAG_BLOCKS = [(t, 2 * t) for t in range(NSLOT - 1, -1, -1)]
BLOCKS += DIAG_BLOCKS
NB = len(BLOCKS)               # 20
NPAIRED = NB - len(DIAG_BLOCKS)  # 16 blocks -> 8 colsum pairs

_COMPILED = None
_last_bass_results = None


def _build():
    from concourse import bacc, mybir
    import concourse.tile as tile

    f32 = mybir.dt.float32
    bf16 = mybir.dt.bfloat16
    fp8 = mybir.dt.float8e4
    DR = mybir.MatmulPerfMode.DoubleRow
    nc = bacc.Bacc("TRN2", target_bir_lowering=False, debug=False,
                   num_devices=NCORES)

    # rows: per-core lhsT data, [128, chunk, slot*RTP + row] fp8.
    # chunks 0-1 = normalized feat rows, chunks 2-7 = NEGATED normalized
    # frozen; all pre-scaled by FP8_SCALE.
    rows = nc.declare_dram_parameter("rows", [128, NK, NSLOT * RTP], fp8,
                                     isOutput=False)
    # bands[J]: all 8 chunks' columns [392J, 392J+392) of the full scaled
    # normalized (transposed) matrices, [128, chunk, col(+pad)] per J.
    bands = nc.declare_dram_parameter("bands", [NJ, 128, NK * CBP], fp8,
                                      isOutput=False)
    # cmask[:, b, :NJ]: one-hot row-mask for block b's column-sum matmul;
    # [98, NB, 16] fp8 so block pairs form a DoubleRow weight [98, 2, 8].
    cmask = nc.declare_dram_parameter("cmask", [RT, NB, 16], fp8,
                                      isOutput=False)
    racc_out = nc.declare_dram_parameter("racc", [RT, NSLOT * NJ], f32,
                                         isOutput=True)
    cs_out = nc.declare_dram_parameter("cs", [NJ, CB], f32, isOutput=True)

    with tile.TileContext(nc) as tc:
        with (
            tc.tile_pool(name="inp", bufs=1) as inp,
            tc.tile_pool(name="bandp", bufs=4) as bandp,
            tc.tile_pool(name="pd", bufs=6, space="PSUM") as pd,
            tc.tile_pool(name="pw", bufs=1, space="PSUM") as pw,
            tc.tile_pool(name="pcs", bufs=1, space="PSUM") as pcs,
            tc.tile_pool(name="adp", bufs=1) as adp,
            tc.tile_pool(name="outp", bufs=1) as outp,
        ):
            # PE warm-up: trip the HAM clock gate during the DMA wait
            warm_s = inp.tile([128, CB], bf16, name="warm_s", tag="warm_s")
            nc.gpsimd.memset(warm_s[:], 0.0)
            warm_p = pw.tile([128, CB], f32, name="warm_p", tag="warm_p")
            for w in range(8):
                nc.tensor.matmul(warm_p[:], warm_s[:, :128], warm_s[:],
                                 start=True, stop=True)

            rows_t = inp.tile([128, NK, NSLOT * RTP], fp8, name="rows_t",
                              tag="rows_t")
            nc.sync.dma_start(rows_t[:], rows[:])
            # bufs=4: band J+4's DMA gets a WAW dependency on band J's tile
            # being consumed, so only ~4 band transfers compete for HBM at a
            # time and the first bands arrive in priority order.
            band_t = [None] * NJ
            for J in JORDER:
                t_ = bandp.tile([128, NK, CBP], fp8, name=f"band{J}",
                                tag="band")
                if J == JORDER[0]:
                    # split the first band so its first half (chunk pairs
                    # 0-1) lands earlier and the DR stream starts sooner
                    nc.sync.dma_start(t_[:, :NK // 2, :],
                                      bands[J][:, :NK // 2 * CBP])
                    nc.sync.dma_start(t_[:, NK // 2:, :],
                                      bands[J][:, NK // 2 * CBP:])
                else:
                    nc.sync.dma_start(t_[:], bands[J])
                band_t[J] = t_

            cm_t = inp.tile([RT, NB, 16], fp8, name="cm_t", tag="cm_t")
            nc.gpsimd.dma_start(cm_t[:], cmask[:])
            racc_t = outp.tile([RT, NSLOT * NJ], f32, name="racc_t",
                               tag="racc_t")
            nc.gpsimd.memset(racc_t[:], 0.0)
            cs_psum = pcs.tile([NJ, CB], f32, name="cs_psum", tag="cs_psum")

            # DR matmul blocks with column-sum matmuls interleaved two
            # blocks behind: the lag gives ScalarE time to finish Abs before
            # a colsum reaches the front of the Tensor queue. ad is fp8
            # (scaled by 1/4 inside the Abs so |d| <= 512 stays in range) so
            # block PAIRS feed one DoubleRow colsum matmul (10 instead of 20).
            ad_all = adp.tile([RT, NB, CBP], fp8, name="ad_all", tag="ad_all")

            def colsum(q):
                nc.tensor.matmul(
                    cs_psum[:],
                    cm_t[:, 2 * q: 2 * q + 2, :NJ],
                    ad_all[:, 2 * q: 2 * q + 2, :CB],
                    start=(q == 0),
                    stop=(q == NPAIRED // 2 - 1),
                    perf_mode=DR,
                )

            for b, (t, J) in enumerate(BLOCKS):
                d = pd.tile([RT, CB], f32, name=f"d_{t}_{J}", tag="d")
                for p in range(NPAIR):
                    nc.tensor.matmul(
                        d[:],
                        rows_t[:, 2 * p: 2 * p + 2, RTP * t: RTP * t + RT],
                        band_t[J][:, 2 * p: 2 * p + 2, :CB],
                        start=(p == 0),
                        stop=(p == NPAIR - 1),
                        perf_mode=DR,
                    )
                nc.scalar.activation(ad_all[:, b, :CB], d[:],
                                     mybir.ActivationFunctionType.Abs,
                                     scale=0.25)
                nc.vector.tensor_reduce(
                    out=racc_t[:, NSLOT * J + t: NSLOT * J + t + 1],
                    in_=ad_all[:, b, :CB],
                    axis=mybir.AxisListType.X,
                    op=mybir.AluOpType.add,
                )
                if b >= 5 and b % 2 == 1:
                    colsum((b - 5) // 2)

            cs_sb = outp.tile([NJ, CB], f32, name="cs_sb", tag="cs_sb")
            nc.scalar.copy(cs_sb[:], cs_psum[:])
            # separate issue engines so the two output DMA issues overlap
            nc.gpsimd.dma_start(racc_out[:], racc_t[:])
            nc.sync.dma_start(cs_out[:], cs_sb[:])

    nc.compile()
    return nc


def _get_compiled():
    global _COMPILED
    if _COMPILED is None:
        _COMPILED = _build()
    return _COMPILED


def _normalize(x):
    n = np.sqrt((x.astype(np.float64) ** 2).sum(-1, keepdims=True))
    return (x / np.maximum(n, EPS)).astype(np.float32)


def _device_rowsums(fnT, fzT):
    """fnT (256, M), fzT (768, M) f32 -> S (M,) row sums of |feat-frozen|."""
    global _last_bass_results
    from concourse.bass_utils import run_bass_kernel_spmd

    nc = _get_compiled()

    chunks = np.concatenate([fnT.reshape(2, 128, M),
                             fzT.reshape(6, 128, M)], axis=0)  # (8,128,M) f32
    chunks = chunks * FP8_SCALE
    # bands[J, p, 400k + x] = chunks[k, p, 392J + x]
    bands_np = np.zeros((NJ, 128, NK, CBP), np.float32)
    bands_np[:, :, :, :CB] = (
        chunks.reshape(NK, 128, NJ, CB).transpose(2, 1, 0, 3))
    bands_np = np.clip(bands_np, -240.0, 240.0).astype(FP8).reshape(
        NJ, 128, NK * CBP)

    in_maps = []
    for c in range(NCORES):
        rows_np = np.zeros((128, NK, NSLOT * RTP), np.float32)
        for t in range(NSLOT):
            seg = chunks[:, :, RT * (8 * t + c): RT * (8 * t + c) + RT].copy()
            seg[2:] = -seg[2:]                    # negate frozen chunks
            rows_np[:, :, RTP * t: RTP * t + RT] = seg.transpose(1, 0, 2)
        rows_np = np.clip(rows_np, -240.0, 240.0).astype(FP8)
        cm = np.zeros((RT, NB, 16), np.float32)
        for b_, (t, J) in enumerate(BLOCKS):
            if J > 2 * t + c // 4:
                cm[:, b_, J] = 1.0
        in_maps.append({
            "rows": rows_np,
            "bands": bands_np,
            "cmask": cm.astype(FP8),
        })

    res = run_bass_kernel_spmd(nc, in_maps, list(range(NCORES)))
    _last_bass_results = res

    S = np.zeros(M, np.float64)
    for c in range(NCORES):
        racc = res.results[c]["racc"].astype(np.float64)   # (98, 32)
        cs = res.results[c]["cs"].astype(np.float64)       # (8, 392)
        for t in range(NSLOT):
            r = 8 * t + c
            jmin = 2 * t + c // 4
            jinc = [NSLOT * J + t for J in range(max(2 * t, jmin), NJ)]
            S[RT * r: RT * (r + 1)] += racc[:, jinc].sum(1)
        S += cs.reshape(-1)
    # ad carries 0.25*|d| (fp8 range), d carries 256*diff
    return (S * 4.0 / (FP8_SCALE * FP8_SCALE)).astype(np.float32)


def kernel(frozen_embeddings, feature_embeddings, proto_sim, labels):
    fz = np.asarray(frozen_embeddings, dtype=np.float32).reshape(M, D)
    fn = np.asarray(feature_embeddings, dtype=np.float32).reshape(M, NF)
    ps_ = np.asarray(proto_sim, dtype=np.float32)
    lab = np.asarray(labels)

    xnf = _normalize(fn)
    xnz = _normalize(fz)
    fnT = np.ascontiguousarray(xnf.T)
    fzT = np.ascontiguousarray(xnz.T)

    # dense part on the 8 NeuronCores
    S = _device_rowsums(fnT, fzT)

    # prototype max/argmax and labels (host, tiny)
    psr = ps_.transpose(0, 2, 1).reshape(M, P)
    mps = psr.max(1)
    pidx = psr.argmax(1)
    ext = np.repeat(lab, N)

    # sparse ranking candidates: only same-argmax-prototype pairs can be nonzero
    cand_vals, cand_flat = [], []
    for p in np.unique(pidx):
        g = np.nonzero(pidx == p)[0]
        s = len(g)
        if s < 2:
            continue
        F = xnf[g] @ xnf[g].T
        Z = xnz[g] @ xnz[g].T
        V = (F - Z) * np.outer(mps[g], mps[g])
        iu, ju = np.triu_indices(s, 1)
        ok = ext[g][iu] != ext[g][ju]
        if ok.any():
            cand_vals.append(V[iu[ok], ju[ok]].astype(np.float64))
            cand_flat.append(g[iu[ok]].astype(np.int64) * M + g[ju[ok]])
    if cand_vals:
        vals = np.concatenate(cand_vals)
        flats = np.concatenate(cand_flat)
    else:
        vals = np.zeros(0)
        flats = np.zeros(0, np.int64)

    # top-5 with lax.top_k tie semantics (desc value, then asc flat index);
    # entries not in the candidate set are exact zeros in the ranking matrix.
    order = np.lexsort((flats, -vals))
    pos = [f for f in order if vals[f] > 0][:K_]
    sel_flats = [int(flats[i]) for i in pos]
    if len(sel_flats) < K_:
        nonzero = set(int(f) for v, f in zip(vals, flats) if v != 0.0)
        f = 0
        while len(sel_flats) < K_:
            if f not in nonzero:
                sel_flats.append(f)
            f += 1
    sel_flats = np.asarray(sel_flats, np.int64)
    rows = sel_flats // M
    cols = sel_flats % M

    out = GAMMA * (S[rows].sum(dtype=np.float64) + S[cols].sum(dtype=np.float64)) / (2 * K_ * M)
    return np.asarray(np.float32(out))


# revision 31
# speedup vs baseline: 1.1658x; 1.1658x over previous
"""Trainium2 kernel for nn_ConservationOfFeatureSimilarity.

Math (see reference): with xn = row-normalized feature embeddings (M, 256) and
zn = row-normalized frozen embeddings (M, 768), M = B*N = 3136:

  feat_sim  = xn @ xn.T        (M, M)
  frozen_sim= zn @ zn.T        (M, M)
  ranking   = triu+ * (feat-frozen) * [cls_i != cls_j] * [pidx_i == pidx_j] * mps_i*mps_j
  top5      = top_k(ranking.flat, 5);  sel rows/cols
  out       = mean |feat_sim[sel] - frozen_sim[sel]|  over (5, 2, M)
            = (sum over the 10 selected row indices of S[r]) / (10*M)
  where S_i = sum_j |feat_sim[i,j] - frozen_sim[i,j]|.

Device (8 NeuronCores): the dense O(M^2 * D) part — S row sums. |diff| is
symmetric, so only upper-triangular blocks of the (32 x 8) tile grid are
computed: each computed block contributes row sums and, for strictly-upper
blocks, column sums for the mirrored block (ones-masked matmul on |d|).
Per-core work is SPMD-uniform: core c owns row-tiles {8t+c : t=0..3} (98 rows
each) and slot t computes col-blocks J >= 2t (392 cols each); per-core 0/1
mask vectors (data, not code) select which blocks feed the column-sum
accumulator, and the host drops the few below-diagonal rowsum partials.

The tile difference feat-frozen is accumulated directly in PSUM via 4 chained
fp8(e4m3) DoubleRow matmuls (each contracting 2 128-chunks: 1 pair for
+xn.xn, 3 pairs for (-zn).zn using host-negated row slices; inputs pre-scaled
by 16 so quantization stays in fp8 normal range, descaled by 256 on host).
A single fused DVE tensor_scalar(abs_max, accum_out) produces |d| in SBUF
(for the masked column-sum matmul on TensorE) and the row sums in one pass.

Host: normalization/transposes, fp8 quantization, prototype argmax, the
top-5 search (ranking is nonzero only for same-argmax-prototype pairs:
~25K of the 9.8M pairs, evaluated sparsely in numpy in exact arithmetic,
so selection is unaffected by fp8), and the final scalar combine.
"""

import sys

if "/opt/trn_rl_repo" not in sys.path:
    sys.path.insert(0, "/opt/trn_rl_repo")

import numpy as np
import ml_dtypes

BF16 = ml_dtypes.bfloat16
FP8 = ml_dtypes.float8_e4m3

B, N, D, NF, P = 16, 196, 768, 256, 200
M = B * N                      # 3136
NCORES = 8
RT = 98                        # row tile height
RTP = 112                      # row tile slot pitch in SBUF (16B aligned)
NSLOT = 4                      # row tiles per core (slot t -> global tile 8t+c)
CB = 392                       # col block width
CBP = 400                      # col block pitch in SBUF (16B aligned)
NJ = 8                         # col blocks
NK = 8                         # 128-chunks: 2 feat + 6 frozen
NPAIR = 4                      # DoubleRow chunk pairs
K_ = 5
GAMMA = 1.0
EPS = 1e-8
FP8_SCALE = 16.0               # inputs pre-scaled; |d| scaled by 256

# program-order block list: (t, J) with J >= 2t. Descending J so the first
# bands DMA'd in feed the largest block groups (PE consumption stays behind
# the DMA arrival curve). The four diagonal blocks (J == 2t) never feed the
# column-sum accumulator (their cmask is all-zero for every core), so they
# are deferred to the end of the stream where they use the short fused
# Abs+accum path with no colsum dependency chain behind them.
JORDER = list(range(NJ - 1, -1, -1))
BLOCKS = [(t, J) for J in JORDER for t in range(NSLOT) if J > 2 * t]
DIAG_BLOCKS = [(t, 2 * t) for t in range(NSLOT - 1, -1, -1)]
BLOCKS += DIAG_BLOCKS
NB = len(BLOCKS)               # 20
NPAIRED = NB - len(DIAG_BLOCKS)  # 16 blocks -> 8 colsum pairs

_COMPILED = None
_last_bass_results = None


def _build():
    from concourse import bacc, mybir
    import concourse.tile as tile

    f32 = mybir.dt.float32
    bf16 = mybir.dt.bfloat16
    fp8 = mybir.dt.float8e4
    DR = mybir.MatmulPerfMode.DoubleRow
    nc = bacc.Bacc("TRN2", target_bir_lowering=False, debug=False,
                   num_devices=NCORES)

    # rows: per-core lhsT data, [128, chunk, slot*RTP + row] fp8.
    # chunks 0-1 = normalized feat rows, chunks 2-7 = NEGATED normalized
    # frozen; all pre-scaled by FP8_SCALE.
    rows = nc.declare_dram_parameter("rows", [128, NK, NSLOT * RTP], fp8,
                                     isOutput=False)
    # bands[J]: all 8 chunks' columns [392J, 392J+392) of the full scaled
    # normalized (transposed) matrices, [128, chunk, col(+pad)] per J.
    bands = nc.declare_dram_parameter("bands", [NJ, 128, NK * CBP], fp8,
                                      isOutput=False)
    # cmask[:, b, :NJ]: one-hot row-mask for block b's column-sum matmul;
    # [98, NB, 16] fp8 so block pairs form a DoubleRow weight [98, 2, 8].
    cmask = nc.declare_dram_parameter("cmask", [RT, NB, 16], fp8,
                                      isOutput=False)
    racc_out = nc.declare_dram_parameter("racc", [RT, NSLOT * NJ], f32,
                                         isOutput=True)
    cs_out = nc.declare_dram_parameter("cs", [NJ, CB], f32, isOutput=True)

    with tile.TileContext(nc) as tc:
        with (
            tc.tile_pool(name="inp", bufs=1) as inp,
            tc.tile_pool(name="pd", bufs=6, space="PSUM") as pd,
            tc.tile_pool(name="pw", bufs=1, space="PSUM") as pw,
            tc.tile_pool(name="pcs", bufs=1, space="PSUM") as pcs,
            tc.tile_pool(name="adp", bufs=1) as adp,
            tc.tile_pool(name="outp", bufs=1) as outp,
        ):
            # PE warm-up: trip the HAM clock gate during the DMA wait
            warm_s = inp.tile([128, CB], bf16, name="warm_s", tag="warm_s")
            nc.gpsimd.memset(warm_s[:], 0.0)
            warm_p = pw.tile([128, CB], f32, name="warm_p", tag="warm_p")
            for w in range(8):
                nc.tensor.matmul(warm_p[:], warm_s[:, :128], warm_s[:],
                                 start=True, stop=True)

            rows_t = inp.tile([128, NK, NSLOT * RTP], fp8, name="rows_t",
                              tag="rows_t")
            nc.sync.dma_start(rows_t[:], rows[:])
            band_t = [None] * NJ
            for J in JORDER:
                t_ = inp.tile([128, NK, CBP], fp8, name=f"band{J}",
                              tag=f"band{J}")
                if J == JORDER[0]:
                    # split the first band so its first half (chunk pairs
                    # 0-1) lands earlier and the DR stream starts sooner
                    nc.sync.dma_start(t_[:, :NK // 2, :],
                                      bands[J][:, :NK // 2 * CBP])
                    nc.sync.dma_start(t_[:, NK // 2:, :],
                                      bands[J][:, NK // 2 * CBP:])
                else:
                    nc.sync.dma_start(t_[:], bands[J])
                band_t[J] = t_

            cm_t = inp.tile([RT, NB, 16], fp8, name="cm_t", tag="cm_t")
            nc.gpsimd.dma_start(cm_t[:], cmask[:])
            racc_t = outp.tile([RT, NSLOT * NJ], f32, name="racc_t",
                               tag="racc_t")
            nc.gpsimd.memset(racc_t[:], 0.0)
            cs_psum = pcs.tile([NJ, CB], f32, name="cs_psum", tag="cs_psum")

            # DR matmul blocks with column-sum matmuls interleaved two
            # blocks behind: the lag gives ScalarE time to finish Abs before
            # a colsum reaches the front of the Tensor queue. ad is fp8
            # (scaled by 1/4 inside the Abs so |d| <= 512 stays in range) so
            # block PAIRS feed one DoubleRow colsum matmul (10 instead of 20).
            ad_all = adp.tile([RT, NB, CBP], fp8, name="ad_all", tag="ad_all")

            def colsum(q):
                nc.tensor.matmul(
                    cs_psum[:],
                    cm_t[:, 2 * q: 2 * q + 2, :NJ],
                    ad_all[:, 2 * q: 2 * q + 2, :CB],
                    start=(q == 0),
                    stop=(q == NPAIRED // 2 - 1),
                    perf_mode=DR,
                )

            for b, (t, J) in enumerate(BLOCKS):
                d = pd.tile([RT, CB], f32, name=f"d_{t}_{J}", tag="d")
                for p in range(NPAIR):
                    nc.tensor.matmul(
                        d[:],
                        rows_t[:, 2 * p: 2 * p + 2, RTP * t: RTP * t + RT],
                        band_t[J][:, 2 * p: 2 * p + 2, :CB],
                        start=(p == 0),
                        stop=(p == NPAIR - 1),
                        perf_mode=DR,
                    )
                nc.scalar.activation(ad_all[:, b, :CB], d[:],
                                     mybir.ActivationFunctionType.Abs,
                                     scale=0.25)
                nc.vector.tensor_reduce(
                    out=racc_t[:, NSLOT * J + t: NSLOT * J + t + 1],
                    in_=ad_all[:, b, :CB],
                    axis=mybir.AxisListType.X,
                    op=mybir.AluOpType.add,
                )
                if b >= 5 and b % 2 == 1:
                    colsum((b - 5) // 2)

            cs_sb = outp.tile([NJ, CB], f32, name="cs_sb", tag="cs_sb")
            nc.scalar.copy(cs_sb[:], cs_psum[:])
            # separate issue engines so the two output DMA issues overlap
            nc.gpsimd.dma_start(racc_out[:], racc_t[:])
            nc.sync.dma_start(cs_out[:], cs_sb[:])

    nc.compile()
    return nc


def _get_compiled():
    global _COMPILED
    if _COMPILED is None:
        _COMPILED = _build()
    return _COMPILED


def _normalize(x):
    n = np.sqrt((x.astype(np.float64) ** 2).sum(-1, keepdims=True))
    return (x / np.maximum(n, EPS)).astype(np.float32)


def _device_rowsums(fnT, fzT):
    """fnT (256, M), fzT (768, M) f32 -> S (M,) row sums of |feat-frozen|."""
    global _last_bass_results
    from concourse.bass_utils import run_bass_kernel_spmd

    nc = _get_compiled()

    chunks = np.concatenate([fnT.reshape(2, 128, M),
                             fzT.reshape(6, 128, M)], axis=0)  # (8,128,M) f32
    chunks = chunks * FP8_SCALE
    # bands[J, p, 400k + x] = chunks[k, p, 392J + x]
    bands_np = np.zeros((NJ, 128, NK, CBP), np.float32)
    bands_np[:, :, :, :CB] = (
        chunks.reshape(NK, 128, NJ, CB).transpose(2, 1, 0, 3))
    bands_np = np.clip(bands_np, -240.0, 240.0).astype(FP8).reshape(
        NJ, 128, NK * CBP)

    in_maps = []
    for c in range(NCORES):
        rows_np = np.zeros((128, NK, NSLOT * RTP), np.float32)
        for t in range(NSLOT):
            seg = chunks[:, :, RT * (8 * t + c): RT * (8 * t + c) + RT].copy()
            seg[2:] = -seg[2:]                    # negate frozen chunks
            rows_np[:, :, RTP * t: RTP * t + RT] = seg.transpose(1, 0, 2)
        rows_np = np.clip(rows_np, -240.0, 240.0).astype(FP8)
        cm = np.zeros((RT, NB, 16), np.float32)
        for b_, (t, J) in enumerate(BLOCKS):
            if J > 2 * t + c // 4:
                cm[:, b_, J] = 1.0
        in_maps.append({
            "rows": rows_np,
            "bands": bands_np,
            "cmask": cm.astype(FP8),
        })

    res = run_bass_kernel_spmd(nc, in_maps, list(range(NCORES)))
    _last_bass_results = res

    S = np.zeros(M, np.float64)
    for c in range(NCORES):
        racc = res.results[c]["racc"].astype(np.float64)   # (98, 32)
        cs = res.results[c]["cs"].astype(np.float64)       # (8, 392)
        for t in range(NSLOT):
            r = 8 * t + c
            jmin = 2 * t + c // 4
            jinc = [NSLOT * J + t for J in range(max(2 * t, jmin), NJ)]
            S[RT * r: RT * (r + 1)] += racc[:, jinc].sum(1)
        S += cs.reshape(-1)
    # ad carries 0.25*|d| (fp8 range), d carries 256*diff
    return (S * 4.0 / (FP8_SCALE * FP8_SCALE)).astype(np.float32)


def kernel(frozen_embeddings, feature_embeddings, proto_sim, labels):
    fz = np.asarray(frozen_embeddings, dtype=np.float32).reshape(M, D)
    fn = np.asarray(feature_embeddings, dtype=np.float32).reshape(M, NF)
    ps_ = np.asarray(proto_sim, dtype=np.float32)
    lab = np.asarray(labels)

    xnf = _normalize(fn)
    xnz = _normalize(fz)
    fnT = np.ascontiguousarray(xnf.T)
    fzT = np.ascontiguousarray(xnz.T)

    # dense part on the 8 NeuronCores
    S = _device_rowsums(fnT, fzT)

    # prototype max/argmax and labels (host, tiny)
    psr = ps_.transpose(0, 2, 1).reshape(M, P)
    mps = psr.max(1)
    pidx = psr.argmax(1)
    ext = np.repeat(lab, N)

    # sparse ranking candidates: only same-argmax-prototype pairs can be nonzero
    cand_vals, cand_flat = [], []
    for p in np.unique(pidx):
        g = np.nonzero(pidx == p)[0]
        s = len(g)
        if s < 2:
            continue
        F = xnf[g] @ xnf[g].T
        Z = xnz[g] @ xnz[g].T
        V = (F - Z) * np.outer(mps[g], mps[g])
        iu, ju = np.triu_indices(s, 1)
        ok = ext[g][iu] != ext[g][ju]
        if ok.any():
            cand_vals.append(V[iu[ok], ju[ok]].astype(np.float64))
            cand_flat.append(g[iu[ok]].astype(np.int64) * M + g[ju[ok]])
    if cand_vals:
        vals = np.concatenate(cand_vals)
        flats = np.concatenate(cand_flat)
    else:
        vals = np.zeros(0)
        flats = np.zeros(0, np.int64)

    # top-5 with lax.top_k tie semantics (desc value, then asc flat index);
    # entries not in the candidate set are exact zeros in the ranking matrix.
    order = np.lexsort((flats, -vals))
    pos = [f for f in order if vals[f] > 0][:K_]
    sel_flats = [int(flats[i]) for i in pos]
    if len(sel_flats) < K_:
        nonzero = set(int(f) for v, f in zip(vals, flats) if v != 0.0)
        f = 0
        while len(sel_flats) < K_:
            if f not in nonzero:
                sel_flats.append(f)
            f += 1
    sel_flats = np.asarray(sel_flats, np.int64)
    rows = sel_flats // M
    cols = sel_flats % M

    out = GAMMA * (S[rows].sum(dtype=np.float64) + S[cols].sum(dtype=np.float64)) / (2 * K_ * M)
    return np.asarray(np.float32(out))
